# revision 14
# baseline (speedup 1.0000x reference)
"""RWKV-4 style WKV attention (nn_Attention_4234837754291) on 8 TRN2 NeuronCores.

Self-contained Bass/Tile kernel. Sharding: core i -> (batch b = i//2,
D-half h = i%2). Each core runs the full pipeline for its (b, h): time-mix
projections k/v/r (contract full D, produce its DL=512 output channels), the
linear-space WKV scan over T on those channels, the sigmoid gate, and a
partial output projection through its DL rows of W_out.T. The host sums the
two D-half partial outputs per batch.

Engine layout (v3):
 - x pre-transposed host-side to [128, KB, T]; each chunk is one plain DMA
   and the t-1 shift is a column offset.
 - time-mix y = s*x_t + x_{t-1}: per-(p,kb) muls on Scalar (a few on
   Vector), then ONE software-DGE accumulate-DMA per projection adds
   x_{t-1} into the whole y block on the DMA engines (zero compute-engine
   cycles for the adds).
 - wkv tail runs in bf16 using the shifted-state form
     wkv_t = (A_{t-1} + e^u*ekv_t) / (B_{t-1} + e^u*ek_t)
   which has no systematic cancellation, so bf16 scan outputs/operands are
   safe (measured rel ~6e-3). Scan state itself stays fp32 in-engine.
 - output partials written fp16; host sums in f32.
"""
import os
import numpy as np
import ml_dtypes
from contextlib import ExitStack

import concourse.bacc as bacc
import concourse.tile as tile
import concourse.mybir as mybir
from concourse.bass_utils import run_bass_kernel_spmd

F32 = mybir.dt.float32
F16 = mybir.dt.float16
BF16 = mybir.dt.bfloat16
AF = mybir.ActivationFunctionType
OP = mybir.AluOpType

B, T, D = 4, 4096, 1024
DL = 512          # D-half per core
TC = 512          # time chunk
NCORES = 8

# mix adds: "dma" = gpsimd software-DGE accumulate DMA; "vec" = vector TT
MIX_ADD = os.environ.get("KMIXADD", "dma")
# of the 24 per-chunk mix muls, how many run on Vector (rest on Scalar)
N_VEC_MULS = int(os.environ.get("KVECMULS", "4"))
# of the 4 PSUM->SBUF output copies per chunk, how many run on Vector
N_VEC_COPIES = int(os.environ.get("KVECCOPIES", "0"))

_NC_CACHE = {}


def _build(D_=D, DL_=DL, T_=T, TC_=TC, n_devices=NCORES):
    KB, MB, NCH = D_ // 128, DL_ // 128, T_ // TC_
    TB = TC_ // 128
    NW = min(512, D_)
    NH = D_ // NW

    nc = bacc.Bacc("TRN2", target_bir_lowering=False, debug=False,
                   num_devices=n_devices)
    # x pre-transposed host-side: [p, kb, t] with input channel d = kb*128+p
    x = nc.dram_tensor("x", (128, KB, T_), BF16, kind="ExternalInput").ap()
    wk = nc.dram_tensor("wk", (D_, DL_), BF16, kind="ExternalInput").ap()
    wv = nc.dram_tensor("wv", (D_, DL_), BF16, kind="ExternalInput").ap()
    wr = nc.dram_tensor("wr", (D_, DL_), BF16, kind="ExternalInput").ap()
    wo = nc.dram_tensor("wo", (DL_, D_), BF16, kind="ExternalInput").ap()
    smix = nc.dram_tensor("smix", (128, 3 * KB), F32, kind="ExternalInput").ap()
    euv = nc.dram_tensor("euv", (128, MB), F32, kind="ExternalInput").ap()
    ewb = nc.dram_tensor("ewb", (128, MB), F32, kind="ExternalInput").ap()
    out = nc.dram_tensor("out", (T_, D_), F16, kind="ExternalOutput").ap()

    with tile.TileContext(nc) as tc, ExitStack() as ctx:
        wpool = ctx.enter_context(tc.tile_pool(name="weights", bufs=1))
        xt_pool = ctx.enter_context(tc.tile_pool(name="xt", bufs=2))
        y_pool = ctx.enter_context(tc.tile_pool(name="y", bufs=2))
        pp_pool = ctx.enter_context(tc.tile_pool(name="pp", bufs=4, space="PSUM"))
        po_pool = ctx.enter_context(tc.tile_pool(name="po", bufs=1, space="PSUM"))
        ee_pool = ctx.enter_context(tc.tile_pool(name="ee", bufs=2))
        ab_pool = ctx.enter_context(tc.tile_pool(name="ab", bufs=2))
        tr_pool = ctx.enter_context(tc.tile_pool(name="tr", bufs=2))
        ws_pool = ctx.enter_context(tc.tile_pool(name="ws", bufs=2))
        ob_pool = ctx.enter_context(tc.tile_pool(name="ob", bufs=2))
        wkv_pool = ctx.enter_context(tc.tile_pool(name="wkv", bufs=2))
        nd_pool = ctx.enter_context(tc.tile_pool(name="nd", bufs=2))

        def hot(inst, boost=600):
            if inst is not None and inst.ins.bass_priority is not None:
                inst.ins.bass_priority -= boost
            return inst

        # --- startup-critical small DMAs first ---
        smix_sb = wpool.tile([128, 3 * KB], F32, tag="smix")
        hot(nc.sync.dma_start(smix_sb[:], smix[:]), 20000)
        euv_sb = wpool.tile([128, MB], F32, tag="euv")
        hot(nc.sync.dma_start(euv_sb[:], euv[:]), 20000)
        ewb_sb = wpool.tile([128, MB], F32, tag="ewb")
        hot(nc.sync.dma_start(ewb_sb[:], ewb[:]), 20000)

        # chunk-0 x load before the weights, split per-kb so the first mix
        # mul can start as soon as kb0 lands
        xt0 = xt_pool.tile([128, KB, TC_ + 1], BF16, tag="xt")
        nc.gpsimd.memset(xt0[:, :, 0:1], 0.0)
        for kb in range(KB):
            hot(nc.sync.dma_start(xt0[:, kb, 1:TC_ + 1], x[:, kb, 0:TC_]),
                15000 - 10 * kb)

        wk_sb, wv_sb, wr_sb = [], [], []
        for kb in range(KB):
            for lst, src, nm, pr in ((wk_sb, wk, "wk", 9000),
                                     (wv_sb, wv, "wv", 8000),
                                     (wr_sb, wr, "wr", 7000)):
                t = wpool.tile([128, DL_], BF16, tag=f"{nm}{kb}")
                hot(nc.sync.dma_start(t[:], src[kb * 128:(kb + 1) * 128, :]), pr)
                lst.append(t)
        wo_sb = []
        for mb in range(MB):
            t = wpool.tile([128, D_], BF16, tag=f"wo{mb}")
            nc.sync.dma_start(t[:], wo[mb * 128:(mb + 1) * 128, :])
            wo_sb.append(t)

        prevab = [None] * MB
        for c in range(NCH):
            t0 = c * TC_
            if c == 0:
                xt = xt0
            else:
                xt = xt_pool.tile([128, KB, TC_ + 1], BF16, tag="xt")
                nc.sync.dma_start(xt[:], x[:, :, t0 - 1:t0 + TC_])

            # y block [128, 3, KB, TC]: muls write s_p*x_t, then one
            # accumulate-DMA per projection adds x_{t-1} on the DMA engines
            yb = y_pool.tile([128, 3, KB, TC_], BF16, tag="yb")
            nmul = 0
            for pi in range(3):
                for kb in range(KB):
                    sc = smix_sb[:, pi * KB + kb: pi * KB + kb + 1]
                    if nmul < N_VEC_MULS:
                        hot(nc.vector.tensor_scalar(
                            yb[:, pi, kb, :], xt[:, kb, 1:TC_ + 1], sc, None,
                            OP.mult), 300)
                    else:
                        hot(nc.scalar.mul(yb[:, pi, kb, :],
                                          xt[:, kb, 1:TC_ + 1], sc), 300)
                    nmul += 1
                if MIX_ADD == "dma":
                    hot(nc.gpsimd.dma_start(yb[:, pi, :, :], xt[:, :, 0:TC_],
                                            accum_op=OP.add), 300)
                else:
                    for kb in range(KB):
                        hot(nc.vector.tensor_tensor(
                            yb[:, pi, kb, :], yb[:, pi, kb, :],
                            xt[:, kb, 0:TC_], OP.add), 300)

            wss = []
            for mb in range(MB):
                kp = pp_pool.tile([128, TC_], F32, tag="pp")
                for kb in range(KB):
                    nc.tensor.matmul(kp[:], wk_sb[kb][:, mb * 128:(mb + 1) * 128],
                                     yb[:, 0, kb, :], start=(kb == 0),
                                     stop=(kb == KB - 1))
                vp = pp_pool.tile([128, TC_], F32, tag="pp")
                for kb in range(KB):
                    nc.tensor.matmul(vp[:], wv_sb[kb][:, mb * 128:(mb + 1) * 128],
                                     yb[:, 1, kb, :], start=(kb == 0),
                                     stop=(kb == KB - 1))
                rp = pp_pool.tile([128, TC_], F32, tag="pp")
                for kb in range(KB):
                    nc.tensor.matmul(rp[:], wr_sb[kb][:, mb * 128:(mb + 1) * 128],
                                     yb[:, 2, kb, :], start=(kb == 0),
                                     stop=(kb == KB - 1))

                # ee = [ekv | ek] bf16; ab = [A | B] bf16 with leading carry col
                ee = ee_pool.tile([128, 2, TC_], BF16, tag=f"ee{mb}")
                hot(nc.scalar.activation(ee[:, 1, :], kp[:], AF.Exp))
                # sigmoid(r) = (1+tanh(r/2))/2, 0.5 folded into wo host-side;
                # tanh and exp share one activation table
                tr = tr_pool.tile([128, TC_], BF16, tag=f"tr{mb}")
                hot(nc.scalar.activation(tr[:], rp[:], AF.Tanh, scale=0.5))
                hot(nc.scalar.add(tr[:], tr[:], 1.0))
                hot(nc.vector.tensor_tensor(ee[:, 0, :], ee[:, 1, :],
                                            vp[:], OP.mult))

                ab = ab_pool.tile([128, 2, TC_ + 1], BF16, tag=f"ab{mb}")
                if c == 0:
                    nc.gpsimd.memset(ab[:, :, 0:1], 0.0)
                    initA, initB = 0.0, 0.0
                else:
                    # carry col 0 <- prev chunk's last A/B; also seeds scans
                    hot(nc.scalar.copy(ab[:, :, 0:1],
                                       prevab[mb][:, :, TC_:TC_ + 1]))
                    initA = prevab[mb][:, 0, TC_:TC_ + 1]
                    initB = prevab[mb][:, 1, TC_:TC_ + 1]
                ewbc = ewb_sb[:, mb:mb + 1].to_broadcast([128, TC_])
                hot(nc.vector.tensor_tensor_scan(ab[:, 0, 1:TC_ + 1], ewbc,
                                                 ee[:, 0, :], initA,
                                                 OP.mult, OP.add))
                hot(nc.vector.tensor_tensor_scan(ab[:, 1, 1:TC_ + 1], ewbc,
                                                 ee[:, 1, :], initB,
                                                 OP.mult, OP.add))
                prevab[mb] = ab

                eus = euv_sb[:, mb:mb + 1]
                # [numer | den] in one fused STT over both halves (f32 out;
                # reciprocal_approx needs fp32 bit layout):
                # numer = eu*ekv + A_{t-1}, den = eu*ek + B_{t-1}
                nd = nd_pool.tile([128, 2, TC_], F32, tag=f"nd{mb}")
                hot(nc.vector.scalar_tensor_tensor(nd[:], ee[:], eus,
                                                   ab[:, :, 0:TC_],
                                                   OP.mult, OP.add))
                hot(nc.vector.reciprocal_approx_fast(nd[:, 1, :], nd[:, 1, :]))
                wkv = wkv_pool.tile([128, TC_], BF16, tag=f"wkv{mb}")
                hot(nc.vector.tensor_tensor(wkv[:], nd[:, 0, :],
                                            nd[:, 1, :], OP.mult))
                ws = ws_pool.tile([128, TC_], BF16, tag=f"ws{mb}")
                hot(nc.vector.tensor_tensor(ws[:], tr[:], wkv[:], OP.mult))
                wss.append(ws)

            ncp = 0
            for pair in range(TB // 2):
                pos = [po_pool.tile([128, D_], F32, tag=f"po{i}", name=f"po{i}") for i in range(2)]
                for mb in range(MB):
                    for i, tb in enumerate((pair * 2, pair * 2 + 1)):
                        for half in range(NH):
                            nc.tensor.matmul(
                                pos[i][:, half * NW:(half + 1) * NW],
                                wss[mb][:, tb * 128:(tb + 1) * 128],
                                wo_sb[mb][:, half * NW:(half + 1) * NW],
                                start=(mb == 0), stop=(mb == MB - 1))
                for i, tb in enumerate((pair * 2, pair * 2 + 1)):
                    ob = ob_pool.tile([128, D_], F16, tag="ob")
                    if ncp < N_VEC_COPIES:
                        hot(nc.vector.tensor_copy(ob[:], pos[i][:]))
                    else:
                        hot(nc.scalar.copy(ob[:], pos[i][:]))
                    ncp += 1
                    nc.sync.dma_start(out[t0 + tb * 128:t0 + (tb + 1) * 128, :],
                                      ob[:])

    nc.compile()
    return nc


def get_nc():
    if "nc" not in _NC_CACHE:
        _NC_CACHE["nc"] = _build()
    return _NC_CACHE["nc"]


def make_in_maps(x, time_decay, time_first, time_mix_k, time_mix_v, time_mix_r,
                 W_key, W_value, W_receptance, W_output):
    x = np.asarray(x, np.float32)
    time_decay = np.asarray(time_decay, np.float64)
    time_first = np.asarray(time_first, np.float64)
    mk = np.asarray(time_mix_k, np.float64).reshape(-1)
    mv = np.asarray(time_mix_v, np.float64).reshape(-1)
    mr = np.asarray(time_mix_r, np.float64).reshape(-1)
    W_key = np.asarray(W_key, np.float32)
    W_value = np.asarray(W_value, np.float32)
    W_receptance = np.asarray(W_receptance, np.float32)
    W_output = np.asarray(W_output, np.float32)

    KB = D // 128
    ew = np.exp(-np.exp(time_decay)).astype(np.float32)
    eu = np.exp(time_first).astype(np.float32)

    def blocked(vec, nb):
        return np.ascontiguousarray(vec.reshape(nb, 128).T.astype(np.float32))

    smix = np.concatenate([blocked((m / (1.0 - m)), KB) for m in (mk, mv, mr)],
                          axis=1)

    halves = []
    for h in range(2):
        dsl = slice(h * DL, (h + 1) * DL)
        MB = DL // 128

        def eff_w(W, m):
            return np.ascontiguousarray(
                ((1.0 - m)[:, None] * W.T[:, dsl])).astype(ml_dtypes.bfloat16)

        halves.append({
            "wk": eff_w(W_key, mk),
            "wv": eff_w(W_value, mv),
            "wr": eff_w(W_receptance, mr),
            "wo": np.ascontiguousarray(0.5 * W_output.T[dsl, :]).astype(
                ml_dtypes.bfloat16),
            "smix": smix,
            "euv": blocked(eu[dsl], MB),
            "ewb": blocked(ew[dsl], MB),
        })

    in_maps = []
    for i in range(NCORES):
        b, h = i // 2, i % 2
        m = dict(halves[h])
        # [p, kb, t] layout: channel d = kb*128 + p
        m["x"] = np.ascontiguousarray(
            x[b].T.reshape(KB, 128, T).transpose(1, 0, 2)).astype(
                ml_dtypes.bfloat16)
        in_maps.append(m)
    return in_maps


def run(in_maps, trace=False):
    nc = get_nc()
    return run_bass_kernel_spmd(nc, in_maps, core_ids=list(range(NCORES)),
                                trace=trace)


def kernel(**inputs):
    in_maps = make_in_maps(**inputs)
    res = run(in_maps, trace=bool(int(os.environ.get("KERNEL_TRACE", "0"))))
    out = np.zeros((B, T, D), np.float32)
    for i in range(NCORES):
        out[i // 2] += np.asarray(res.results[i]["out"], np.float32)
    if res.exec_time_ns is not None:
        print(f"HW exec time: {res.exec_time_ns} ns")
    return out


# revision 15
# speedup vs baseline: 1.1381x; 1.1381x over previous
"""RWKV-4 style WKV attention (nn_Attention_4234837754291) on 8 TRN2 NeuronCores.

Self-contained Bass/Tile kernel. Sharding: core i -> (batch b = i//2,
D-half h = i%2). Each core runs the full pipeline for its (b, h): time-mix
projections k/v/r (contract full D, produce its DL=512 output channels), the
linear-space WKV scan over T on those channels, the sigmoid gate, and a
partial output projection through its DL rows of W_out.T. The host sums the
two D-half partial outputs per batch.

Engine layout (v3):
 - x pre-transposed host-side to [128, KB, T]; each chunk is one plain DMA
   and the t-1 shift is a column offset.
 - time-mix y = s*x_t + x_{t-1}: per-(p,kb) muls on Scalar (a few on
   Vector), then ONE software-DGE accumulate-DMA per projection adds
   x_{t-1} into the whole y block on the DMA engines (zero compute-engine
   cycles for the adds).
 - wkv tail runs in bf16 using the shifted-state form
     wkv_t = (A_{t-1} + e^u*ekv_t) / (B_{t-1} + e^u*ek_t)
   which has no systematic cancellation, so bf16 scan outputs/operands are
   safe (measured rel ~6e-3). Scan state itself stays fp32 in-engine.
 - output partials written fp16; host sums in f32.
"""
import os
import numpy as np
import ml_dtypes
from contextlib import ExitStack

import concourse.bacc as bacc
import concourse.tile as tile
import concourse.mybir as mybir
from concourse.bass_utils import run_bass_kernel_spmd

F32 = mybir.dt.float32
F16 = mybir.dt.float16
BF16 = mybir.dt.bfloat16
AF = mybir.ActivationFunctionType
OP = mybir.AluOpType

B, T, D = 4, 4096, 1024
DL = 512          # D-half per core
TC = 512          # time chunk
NCORES = 8

# mix adds: "dma" = gpsimd software-DGE accumulate DMA (measured ~9 GB/s,
# too slow); "vec" = vector TT
MIX_ADD = os.environ.get("KMIXADD", "vec")
# of the 24 per-chunk mix muls, how many run on Vector (rest on Scalar)
N_VEC_MULS = int(os.environ.get("KVECMULS", "4"))
# of the 4 PSUM->SBUF output copies per chunk, how many run on Vector
N_VEC_COPIES = int(os.environ.get("KVECCOPIES", "0"))

_NC_CACHE = {}


def _build(D_=D, DL_=DL, T_=T, TC_=TC, n_devices=NCORES):
    KB, MB, NCH = D_ // 128, DL_ // 128, T_ // TC_
    TB = TC_ // 128
    NW = min(512, D_)
    NH = D_ // NW

    nc = bacc.Bacc("TRN2", target_bir_lowering=False, debug=False,
                   num_devices=n_devices)
    # x pre-transposed host-side: [p, kb, t] with input channel d = kb*128+p
    x = nc.dram_tensor("x", (128, KB, T_), BF16, kind="ExternalInput").ap()
    wk = nc.dram_tensor("wk", (D_, DL_), BF16, kind="ExternalInput").ap()
    wv = nc.dram_tensor("wv", (D_, DL_), BF16, kind="ExternalInput").ap()
    wr = nc.dram_tensor("wr", (D_, DL_), BF16, kind="ExternalInput").ap()
    wo = nc.dram_tensor("wo", (DL_, D_), BF16, kind="ExternalInput").ap()
    smix = nc.dram_tensor("smix", (128, 3 * KB), F32, kind="ExternalInput").ap()
    euv = nc.dram_tensor("euv", (128, MB), F32, kind="ExternalInput").ap()
    ewb = nc.dram_tensor("ewb", (128, MB), F32, kind="ExternalInput").ap()
    out = nc.dram_tensor("out", (T_, D_), F16, kind="ExternalOutput").ap()

    with tile.TileContext(nc) as tc, ExitStack() as ctx:
        wpool = ctx.enter_context(tc.tile_pool(name="weights", bufs=1))
        xt_pool = ctx.enter_context(tc.tile_pool(name="xt", bufs=2))
        y_pool = ctx.enter_context(tc.tile_pool(name="y", bufs=2))
        pp_pool = ctx.enter_context(tc.tile_pool(name="pp", bufs=4, space="PSUM"))
        po_pool = ctx.enter_context(tc.tile_pool(name="po", bufs=1, space="PSUM"))
        ee_pool = ctx.enter_context(tc.tile_pool(name="ee", bufs=2))
        ab_pool = ctx.enter_context(tc.tile_pool(name="ab", bufs=2))
        tr_pool = ctx.enter_context(tc.tile_pool(name="tr", bufs=2))
        ws_pool = ctx.enter_context(tc.tile_pool(name="ws", bufs=2))
        ob_pool = ctx.enter_context(tc.tile_pool(name="ob", bufs=2))
        wkv_pool = ctx.enter_context(tc.tile_pool(name="wkv", bufs=2))
        nd_pool = ctx.enter_context(tc.tile_pool(name="nd", bufs=2))

        def hot(inst, boost=600):
            if inst is not None and inst.ins.bass_priority is not None:
                inst.ins.bass_priority -= boost
            return inst

        # --- startup-critical small DMAs first ---
        smix_sb = wpool.tile([128, 3 * KB], F32, tag="smix")
        hot(nc.sync.dma_start(smix_sb[:], smix[:]), 20000)
        euv_sb = wpool.tile([128, MB], F32, tag="euv")
        hot(nc.sync.dma_start(euv_sb[:], euv[:]), 20000)
        ewb_sb = wpool.tile([128, MB], F32, tag="ewb")
        hot(nc.sync.dma_start(ewb_sb[:], ewb[:]), 20000)

        # chunk-0 x load before the weights, split per-kb so the first mix
        # mul can start as soon as kb0 lands
        xt0 = xt_pool.tile([128, KB, TC_ + 1], BF16, tag="xt")
        nc.gpsimd.memset(xt0[:, :, 0:1], 0.0)
        for kb in range(KB):
            hot(nc.sync.dma_start(xt0[:, kb, 1:TC_ + 1], x[:, kb, 0:TC_]),
                15000 - 10 * kb)

        wk_sb, wv_sb, wr_sb = [], [], []
        for kb in range(KB):
            for lst, src, nm, pr in ((wk_sb, wk, "wk", 9000),
                                     (wv_sb, wv, "wv", 8000),
                                     (wr_sb, wr, "wr", 7000)):
                t = wpool.tile([128, DL_], BF16, tag=f"{nm}{kb}")
                hot(nc.sync.dma_start(t[:], src[kb * 128:(kb + 1) * 128, :]), pr)
                lst.append(t)
        wo_sb = []
        for mb in range(MB):
            t = wpool.tile([128, D_], BF16, tag=f"wo{mb}")
            nc.sync.dma_start(t[:], wo[mb * 128:(mb + 1) * 128, :])
            wo_sb.append(t)

        prevab = [None] * MB
        for c in range(NCH):
            t0 = c * TC_
            if c == 0:
                xt = xt0
            else:
                xt = xt_pool.tile([128, KB, TC_ + 1], BF16, tag="xt")
                nc.sync.dma_start(xt[:], x[:, :, t0 - 1:t0 + TC_])

            # y block [128, 3, KB, TC]: muls write s_p*x_t, then one
            # accumulate-DMA per projection adds x_{t-1} on the DMA engines
            yb = y_pool.tile([128, 3, KB, TC_], BF16, tag="yb")
            nmul = 0
            for pi in range(3):
                for kb in range(KB):
                    sc = smix_sb[:, pi * KB + kb: pi * KB + kb + 1]
                    if nmul < N_VEC_MULS:
                        hot(nc.vector.tensor_scalar(
                            yb[:, pi, kb, :], xt[:, kb, 1:TC_ + 1], sc, None,
                            OP.mult), 300)
                    else:
                        hot(nc.scalar.mul(yb[:, pi, kb, :],
                                          xt[:, kb, 1:TC_ + 1], sc), 300)
                    nmul += 1
                if MIX_ADD == "dma":
                    hot(nc.gpsimd.dma_start(yb[:, pi, :, :], xt[:, :, 0:TC_],
                                            accum_op=OP.add), 300)
                else:
                    for kb in range(KB):
                        hot(nc.vector.tensor_tensor(
                            yb[:, pi, kb, :], yb[:, pi, kb, :],
                            xt[:, kb, 0:TC_], OP.add), 300)

            wss = []
            for mb in range(MB):
                kp = pp_pool.tile([128, TC_], F32, tag="pp")
                for kb in range(KB):
                    nc.tensor.matmul(kp[:], wk_sb[kb][:, mb * 128:(mb + 1) * 128],
                                     yb[:, 0, kb, :], start=(kb == 0),
                                     stop=(kb == KB - 1))
                vp = pp_pool.tile([128, TC_], F32, tag="pp")
                for kb in range(KB):
                    nc.tensor.matmul(vp[:], wv_sb[kb][:, mb * 128:(mb + 1) * 128],
                                     yb[:, 1, kb, :], start=(kb == 0),
                                     stop=(kb == KB - 1))
                rp = pp_pool.tile([128, TC_], F32, tag="pp")
                for kb in range(KB):
                    nc.tensor.matmul(rp[:], wr_sb[kb][:, mb * 128:(mb + 1) * 128],
                                     yb[:, 2, kb, :], start=(kb == 0),
                                     stop=(kb == KB - 1))

                # ee = [ekv | ek] bf16; ab = [A | B] bf16 with leading carry col
                ee = ee_pool.tile([128, 2, TC_], BF16, tag=f"ee{mb}")
                hot(nc.scalar.activation(ee[:, 1, :], kp[:], AF.Exp))
                # sigmoid(r) = (1+tanh(r/2))/2, 0.5 folded into wo host-side;
                # tanh and exp share one activation table
                tr = tr_pool.tile([128, TC_], BF16, tag=f"tr{mb}")
                hot(nc.scalar.activation(tr[:], rp[:], AF.Tanh, scale=0.5))
                hot(nc.scalar.add(tr[:], tr[:], 1.0))
                hot(nc.vector.tensor_tensor(ee[:, 0, :], ee[:, 1, :],
                                            vp[:], OP.mult))

                ab = ab_pool.tile([128, 2, TC_ + 1], BF16, tag=f"ab{mb}")
                if c == 0:
                    nc.gpsimd.memset(ab[:, :, 0:1], 0.0)
                    initA, initB = 0.0, 0.0
                else:
                    # carry col 0 <- prev chunk's last A/B; also seeds scans
                    hot(nc.scalar.copy(ab[:, :, 0:1],
                                       prevab[mb][:, :, TC_:TC_ + 1]))
                    initA = prevab[mb][:, 0, TC_:TC_ + 1]
                    initB = prevab[mb][:, 1, TC_:TC_ + 1]
                ewbc = ewb_sb[:, mb:mb + 1].to_broadcast([128, TC_])
                hot(nc.vector.tensor_tensor_scan(ab[:, 0, 1:TC_ + 1], ewbc,
                                                 ee[:, 0, :], initA,
                                                 OP.mult, OP.add))
                hot(nc.vector.tensor_tensor_scan(ab[:, 1, 1:TC_ + 1], ewbc,
                                                 ee[:, 1, :], initB,
                                                 OP.mult, OP.add))
                prevab[mb] = ab

                eus = euv_sb[:, mb:mb + 1]
                # [numer | den] in one fused STT over both halves (f32 out;
                # reciprocal_approx needs fp32 bit layout):
                # numer = eu*ekv + A_{t-1}, den = eu*ek + B_{t-1}
                nd = nd_pool.tile([128, 2, TC_], F32, tag=f"nd{mb}")
                hot(nc.vector.scalar_tensor_tensor(nd[:], ee[:], eus,
                                                   ab[:, :, 0:TC_],
                                                   OP.mult, OP.add))
                hot(nc.vector.reciprocal_approx_fast(nd[:, 1, :], nd[:, 1, :]))
                wkv = wkv_pool.tile([128, TC_], BF16, tag=f"wkv{mb}")
                hot(nc.vector.tensor_tensor(wkv[:], nd[:, 0, :],
                                            nd[:, 1, :], OP.mult))
                ws = ws_pool.tile([128, TC_], BF16, tag=f"ws{mb}")
                hot(nc.vector.tensor_tensor(ws[:], tr[:], wkv[:], OP.mult))
                wss.append(ws)

            ncp = 0
            for pair in range(TB // 2):
                pos = [po_pool.tile([128, D_], F32, tag=f"po{i}", name=f"po{i}") for i in range(2)]
                for mb in range(MB):
                    for i, tb in enumerate((pair * 2, pair * 2 + 1)):
                        for half in range(NH):
                            nc.tensor.matmul(
                                pos[i][:, half * NW:(half + 1) * NW],
                                wss[mb][:, tb * 128:(tb + 1) * 128],
                                wo_sb[mb][:, half * NW:(half + 1) * NW],
                                start=(mb == 0), stop=(mb == MB - 1))
                for i, tb in enumerate((pair * 2, pair * 2 + 1)):
                    ob = ob_pool.tile([128, D_], F16, tag="ob")
                    if ncp < N_VEC_COPIES:
                        hot(nc.vector.tensor_copy(ob[:], pos[i][:]))
                    else:
                        hot(nc.scalar.copy(ob[:], pos[i][:]))
                    ncp += 1
                    nc.sync.dma_start(out[t0 + tb * 128:t0 + (tb + 1) * 128, :],
                                      ob[:])

    nc.compile()
    return nc


def get_nc():
    if "nc" not in _NC_CACHE:
        _NC_CACHE["nc"] = _build()
    return _NC_CACHE["nc"]


def make_in_maps(x, time_decay, time_first, time_mix_k, time_mix_v, time_mix_r,
                 W_key, W_value, W_receptance, W_output):
    x = np.asarray(x, np.float32)
    time_decay = np.asarray(time_decay, np.float64)
    time_first = np.asarray(time_first, np.float64)
    mk = np.asarray(time_mix_k, np.float64).reshape(-1)
    mv = np.asarray(time_mix_v, np.float64).reshape(-1)
    mr = np.asarray(time_mix_r, np.float64).reshape(-1)
    W_key = np.asarray(W_key, np.float32)
    W_value = np.asarray(W_value, np.float32)
    W_receptance = np.asarray(W_receptance, np.float32)
    W_output = np.asarray(W_output, np.float32)

    KB = D // 128
    ew = np.exp(-np.exp(time_decay)).astype(np.float32)
    eu = np.exp(time_first).astype(np.float32)

    def blocked(vec, nb):
        return np.ascontiguousarray(vec.reshape(nb, 128).T.astype(np.float32))

    smix = np.concatenate([blocked((m / (1.0 - m)), KB) for m in (mk, mv, mr)],
                          axis=1)

    halves = []
    for h in range(2):
        dsl = slice(h * DL, (h + 1) * DL)
        MB = DL // 128

        def eff_w(W, m):
            return np.ascontiguousarray(
                ((1.0 - m)[:, None] * W.T[:, dsl])).astype(ml_dtypes.bfloat16)

        halves.append({
            "wk": eff_w(W_key, mk),
            "wv": eff_w(W_value, mv),
            "wr": eff_w(W_receptance, mr),
            "wo": np.ascontiguousarray(0.5 * W_output.T[dsl, :]).astype(
                ml_dtypes.bfloat16),
            "smix": smix,
            "euv": blocked(eu[dsl], MB),
            "ewb": blocked(ew[dsl], MB),
        })

    in_maps = []
    for i in range(NCORES):
        b, h = i // 2, i % 2
        m = dict(halves[h])
        # [p, kb, t] layout: channel d = kb*128 + p
        m["x"] = np.ascontiguousarray(
            x[b].T.reshape(KB, 128, T).transpose(1, 0, 2)).astype(
                ml_dtypes.bfloat16)
        in_maps.append(m)
    return in_maps


def run(in_maps, trace=False):
    nc = get_nc()
    return run_bass_kernel_spmd(nc, in_maps, core_ids=list(range(NCORES)),
                                trace=trace)


def kernel(**inputs):
    in_maps = make_in_maps(**inputs)
    res = run(in_maps, trace=bool(int(os.environ.get("KERNEL_TRACE", "0"))))
    out = np.zeros((B, T, D), np.float32)
    for i in range(NCORES):
        out[i // 2] += np.asarray(res.results[i]["out"], np.float32)
    if res.exec_time_ns is not None:
        print(f"HW exec time: {res.exec_time_ns} ns")
    return out


# revision 17
# speedup vs baseline: 1.3174x; 1.1576x over previous
"""RWKV-4 style WKV attention (nn_Attention_4234837754291) on 8 TRN2 NeuronCores.

Self-contained Bass/Tile kernel. Sharding: core i -> (batch b = i//2,
D-half h = i%2). Each core runs the full pipeline for its (b, h): time-mix
projections k/v/r (contract full D, produce its DL=512 output channels), the
linear-space WKV scan over T on those channels, the sigmoid gate, and a
partial output projection through its DL rows of W_out.T. The host sums the
two D-half partial outputs per batch.

v4: baseline compute structure (f32 2D wkv tail — measured fastest DVE op
shapes) plus infra fixes: x pre-transposed host-side to [128, KB, T] (plain
chunk DMAs, no DMA-transpose; the t-1 shift is a column offset), scale
vectors + chunk-0 x DMA'd before the bulk weights (fast start), fp16 output
partials (host sums f32), and the time-mix muls split Scalar/Vector by knob.

Math (linear space; exactly equivalent to the reference's log-space scan):
  y_p[t] = s_p * x[t] + x[t-1]         with s_p = mix_p/(1-mix_p); the (1-mix_p)
                                       factor is folded into the weights
  k = y_k @ Wk_eff, v = y_v @ Wv_eff, r = y_r @ Wr_eff      (bf16 matmuls)
  ek = exp(k)
  A_t = ew*A_{t-1} + ek_t*v_t ;  B_t = ew*B_{t-1} + ek_t    (ew = exp(-exp(td)))
  wkv_t = (A_t + c*ekv_t) / (B_t + c*ek_t)                  with c = ew*e^u - 1
  out = (wkv * (1 + tanh(r/2))) @ (0.5 * W_out.T[dsl])      (sigmoid fold)
"""
import os
import numpy as np
import ml_dtypes
from contextlib import ExitStack

import concourse.bacc as bacc
import concourse.tile as tile
import concourse.mybir as mybir
from concourse.bass_utils import run_bass_kernel_spmd

F32 = mybir.dt.float32
F16 = mybir.dt.float16
BF16 = mybir.dt.bfloat16
AF = mybir.ActivationFunctionType
OP = mybir.AluOpType

B, T, D = 4, 4096, 1024
DL = 512          # D-half per core
TC = 512          # time chunk
NCORES = 8

# of the 24 per-chunk time-mix muls, how many run on Vector via
# tensor_scalar (measured ~434 ns, 2x mode); the rest on Scalar (~805 ns)
N_VEC_MULS = int(os.environ.get("KVECMULS", "8"))
# of the 4 PSUM->SBUF fp16 output copies per chunk, how many on Vector
N_VEC_COPIES = int(os.environ.get("KVECCOPIES", "0"))

_NC_CACHE = {}


def _build(D_=D, DL_=DL, T_=T, TC_=TC, n_devices=NCORES):
    KB, MB, NCH = D_ // 128, DL_ // 128, T_ // TC_
    TB = TC_ // 128
    NW = min(512, D_)
    NH = D_ // NW

    nc = bacc.Bacc("TRN2", target_bir_lowering=False, debug=False,
                   num_devices=n_devices)
    # x pre-transposed host-side: [p, kb, t] with input channel d = kb*128+p
    x = nc.dram_tensor("x", (128, KB, T_), BF16, kind="ExternalInput").ap()
    wk = nc.dram_tensor("wk", (D_, DL_), BF16, kind="ExternalInput").ap()
    wv = nc.dram_tensor("wv", (D_, DL_), BF16, kind="ExternalInput").ap()
    wr = nc.dram_tensor("wr", (D_, DL_), BF16, kind="ExternalInput").ap()
    wo = nc.dram_tensor("wo", (DL_, D_), BF16, kind="ExternalInput").ap()
    smix = nc.dram_tensor("smix", (128, 3 * KB), F32, kind="ExternalInput").ap()
    cvec = nc.dram_tensor("cvec", (128, MB), F32, kind="ExternalInput").ap()
    ewb = nc.dram_tensor("ewb", (128, MB), F32, kind="ExternalInput").ap()
    out = nc.dram_tensor("out", (T_, D_), F16, kind="ExternalOutput").ap()

    with tile.TileContext(nc) as tc, ExitStack() as ctx:
        wpool = ctx.enter_context(tc.tile_pool(name="weights", bufs=1))
        xt_pool = ctx.enter_context(tc.tile_pool(name="xt", bufs=2))
        y_pool = ctx.enter_context(tc.tile_pool(name="y", bufs=2))
        tmp_pool = ctx.enter_context(tc.tile_pool(name="tmp", bufs=2))
        pp_pool = ctx.enter_context(tc.tile_pool(name="pp", bufs=4, space="PSUM"))
        po_pool = ctx.enter_context(tc.tile_pool(name="po", bufs=1, space="PSUM"))
        ee_pool = ctx.enter_context(tc.tile_pool(name="ee", bufs=2))
        ab_pool = ctx.enter_context(tc.tile_pool(name="ab", bufs=2))
        tr_pool = ctx.enter_context(tc.tile_pool(name="tr", bufs=2))
        ws_pool = ctx.enter_context(tc.tile_pool(name="ws", bufs=2))
        ob_pool = ctx.enter_context(tc.tile_pool(name="ob", bufs=2))
        wkv_pool = ctx.enter_context(tc.tile_pool(name="wkv", bufs=2))

        def hot(inst, boost=600):
            if inst is not None and inst.ins.bass_priority is not None:
                inst.ins.bass_priority -= boost
            return inst

        # --- startup-critical small DMAs first ---
        smix_sb = wpool.tile([128, 3 * KB], F32, tag="smix")
        hot(nc.sync.dma_start(smix_sb[:], smix[:]), 20000)
        cvec_sb = wpool.tile([128, MB], F32, tag="cvec")
        hot(nc.sync.dma_start(cvec_sb[:], cvec[:]), 20000)
        ewb_sb = wpool.tile([128, MB], F32, tag="ewb")
        hot(nc.sync.dma_start(ewb_sb[:], ewb[:]), 20000)

        # chunk-0 x load before the weights, split per-kb so the first mix
        # mul can start as soon as kb0 lands
        xt0 = xt_pool.tile([128, KB, TC_ + 1], BF16, tag="xt")
        nc.gpsimd.memset(xt0[:, :, 0:1], 0.0)
        for kb in range(KB):
            hot(nc.sync.dma_start(xt0[:, kb, 1:TC_ + 1], x[:, kb, 0:TC_]),
                15000 - 10 * kb)

        wk_sb, wv_sb, wr_sb = [], [], []
        for kb in range(KB):
            for lst, src, nm, pr in ((wk_sb, wk, "wk", 9000),
                                     (wv_sb, wv, "wv", 8000),
                                     (wr_sb, wr, "wr", 7000)):
                t = wpool.tile([128, DL_], BF16, tag=f"{nm}{kb}")
                hot(nc.sync.dma_start(t[:], src[kb * 128:(kb + 1) * 128, :]), pr)
                lst.append(t)
        wo_sb = []
        for mb in range(MB):
            t = wpool.tile([128, D_], BF16, tag=f"wo{mb}")
            nc.sync.dma_start(t[:], wo[mb * 128:(mb + 1) * 128, :])
            wo_sb.append(t)

        prevA = [None] * MB
        prevB = [None] * MB
        for c in range(NCH):
            t0 = c * TC_
            if c == 0:
                xt = xt0
            else:
                xt = xt_pool.tile([128, KB, TC_ + 1], BF16, tag="xt")
                nc.sync.dma_start(xt[:], x[:, :, t0 - 1:t0 + TC_])

            ys = {}
            nmul = 0
            for pi, p in enumerate(("k", "v", "r")):
                ys[p] = []
                for kb in range(KB):
                    y = y_pool.tile([128, TC_], BF16, tag=f"y{p}{kb}")
                    sc = smix_sb[:, pi * KB + kb: pi * KB + kb + 1]
                    tmp = tmp_pool.tile([128, TC_], BF16, tag=f"tmp{kb}")
                    if nmul < N_VEC_MULS:
                        hot(nc.vector.tensor_scalar(tmp[:],
                                                    xt[:, kb, 1:TC_ + 1],
                                                    sc, None, OP.mult), 300)
                    else:
                        hot(nc.scalar.mul(tmp[:], xt[:, kb, 1:TC_ + 1], sc),
                            300)
                    nmul += 1
                    hot(nc.vector.tensor_tensor(y[:], tmp[:],
                                                xt[:, kb, 0:TC_], OP.add), 300)
                    ys[p].append(y)

            wss = []
            for mb in range(MB):
                kp = pp_pool.tile([128, TC_], F32, tag="pp")
                for kb in range(KB):
                    nc.tensor.matmul(kp[:], wk_sb[kb][:, mb * 128:(mb + 1) * 128],
                                     ys["k"][kb][:], start=(kb == 0),
                                     stop=(kb == KB - 1))
                vp = pp_pool.tile([128, TC_], F32, tag="pp")
                for kb in range(KB):
                    nc.tensor.matmul(vp[:], wv_sb[kb][:, mb * 128:(mb + 1) * 128],
                                     ys["v"][kb][:], start=(kb == 0),
                                     stop=(kb == KB - 1))
                rp = pp_pool.tile([128, TC_], F32, tag="pp")
                for kb in range(KB):
                    nc.tensor.matmul(rp[:], wr_sb[kb][:, mb * 128:(mb + 1) * 128],
                                     ys["r"][kb][:], start=(kb == 0),
                                     stop=(kb == KB - 1))

                # EE = [ekv | ek], AB = [As | Bs] double-width tiles
                ee = ee_pool.tile([128, 2 * TC_], F32, tag=f"ee{mb}")
                hot(nc.scalar.activation(ee[:, TC_:2 * TC_], kp[:], AF.Exp))
                tr = tr_pool.tile([128, TC_], BF16, tag=f"tr{mb}")
                hot(nc.scalar.activation(tr[:], rp[:], AF.Tanh, scale=0.5))
                hot(nc.scalar.add(tr[:], tr[:], 1.0))
                hot(nc.vector.tensor_tensor(ee[:, 0:TC_], ee[:, TC_:2 * TC_],
                                            vp[:], OP.mult))

                ab = ab_pool.tile([128, 2 * TC_], F32, tag=f"ab{mb}")
                initA = 0.0 if c == 0 else prevA[mb][:, TC_ - 1:TC_]
                ewbc = ewb_sb[:, mb:mb + 1].to_broadcast([128, TC_])
                hot(nc.vector.tensor_tensor_scan(ab[:, 0:TC_], ewbc,
                                                 ee[:, 0:TC_], initA,
                                                 OP.mult, OP.add))
                initB = 0.0 if c == 0 else prevB[mb][:, 2 * TC_ - 1:2 * TC_]
                hot(nc.vector.tensor_tensor_scan(ab[:, TC_:2 * TC_], ewbc,
                                                 ee[:, TC_:2 * TC_], initB,
                                                 OP.mult, OP.add))
                prevA[mb], prevB[mb] = ab, ab

                cs = cvec_sb[:, mb:mb + 1]
                # [numer | den] -> EE in one fused STT over both halves
                hot(nc.vector.scalar_tensor_tensor(ee[:], ee[:], cs, ab[:],
                                                   OP.mult, OP.add))
                hot(nc.vector.reciprocal_approx_fast(ee[:, TC_:2 * TC_],
                                                     ee[:, TC_:2 * TC_]))
                wkv = wkv_pool.tile([128, TC_], BF16, tag=f"wkv{mb}")
                hot(nc.vector.tensor_tensor(wkv[:], ee[:, 0:TC_],
                                            ee[:, TC_:2 * TC_], OP.mult))
                ws = ws_pool.tile([128, TC_], BF16, tag=f"ws{mb}")
                hot(nc.vector.tensor_tensor(ws[:], tr[:], wkv[:], OP.mult))
                wss.append(ws)

            ncp = 0
            for pair in range(TB // 2):
                pos = [po_pool.tile([128, D_], F32, tag=f"po{i}", name=f"po{i}") for i in range(2)]
                for mb in range(MB):
                    for i, tb in enumerate((pair * 2, pair * 2 + 1)):
                        for half in range(NH):
                            nc.tensor.matmul(
                                pos[i][:, half * NW:(half + 1) * NW],
                                wss[mb][:, tb * 128:(tb + 1) * 128],
                                wo_sb[mb][:, half * NW:(half + 1) * NW],
                                start=(mb == 0), stop=(mb == MB - 1))
                for i, tb in enumerate((pair * 2, pair * 2 + 1)):
                    ob = ob_pool.tile([128, D_], F16, tag="ob")
                    if ncp < N_VEC_COPIES:
                        hot(nc.vector.tensor_copy(ob[:], pos[i][:]))
                    else:
                        hot(nc.scalar.copy(ob[:], pos[i][:]))
                    ncp += 1
                    nc.sync.dma_start(out[t0 + tb * 128:t0 + (tb + 1) * 128, :],
                                      ob[:])

    nc.compile()
    return nc


def get_nc():
    if "nc" not in _NC_CACHE:
        _NC_CACHE["nc"] = _build()
    return _NC_CACHE["nc"]


def make_in_maps(x, time_decay, time_first, time_mix_k, time_mix_v, time_mix_r,
                 W_key, W_value, W_receptance, W_output):
    x = np.asarray(x, np.float32)
    time_decay = np.asarray(time_decay, np.float64)
    time_first = np.asarray(time_first, np.float64)
    mk = np.asarray(time_mix_k, np.float64).reshape(-1)
    mv = np.asarray(time_mix_v, np.float64).reshape(-1)
    mr = np.asarray(time_mix_r, np.float64).reshape(-1)
    W_key = np.asarray(W_key, np.float32)
    W_value = np.asarray(W_value, np.float32)
    W_receptance = np.asarray(W_receptance, np.float32)
    W_output = np.asarray(W_output, np.float32)

    KB = D // 128
    ew = np.exp(-np.exp(time_decay))
    c = (ew * np.exp(time_first) - 1.0).astype(np.float32)
    ew = ew.astype(np.float32)

    def blocked(vec, nb):
        return np.ascontiguousarray(vec.reshape(nb, 128).T.astype(np.float32))

    smix = np.concatenate([blocked((m / (1.0 - m)), KB) for m in (mk, mv, mr)],
                          axis=1)

    halves = []
    for h in range(2):
        dsl = slice(h * DL, (h + 1) * DL)
        MB = DL // 128

        def eff_w(W, m):
            return np.ascontiguousarray(
                ((1.0 - m)[:, None] * W.T[:, dsl])).astype(ml_dtypes.bfloat16)

        halves.append({
            "wk": eff_w(W_key, mk),
            "wv": eff_w(W_value, mv),
            "wr": eff_w(W_receptance, mr),
            "wo": np.ascontiguousarray(0.5 * W_output.T[dsl, :]).astype(
                ml_dtypes.bfloat16),
            "smix": smix,
            "cvec": blocked(c[dsl], MB),
            "ewb": blocked(ew[dsl], MB),
        })

    in_maps = []
    for i in range(NCORES):
        b, h = i // 2, i % 2
        m = dict(halves[h])
        # [p, kb, t] layout: channel d = kb*128 + p
        m["x"] = np.ascontiguousarray(
            x[b].T.reshape(KB, 128, T).transpose(1, 0, 2)).astype(
                ml_dtypes.bfloat16)
        in_maps.append(m)
    return in_maps


def run(in_maps, trace=False):
    nc = get_nc()
    return run_bass_kernel_spmd(nc, in_maps, core_ids=list(range(NCORES)),
                                trace=trace)


def kernel(**inputs):
    in_maps = make_in_maps(**inputs)
    res = run(in_maps, trace=bool(int(os.environ.get("KERNEL_TRACE", "0"))))
    out = np.zeros((B, T, D), np.float32)
    for i in range(NCORES):
        out[i // 2] += np.asarray(res.results[i]["out"], np.float32)
    if res.exec_time_ns is not None:
        print(f"HW exec time: {res.exec_time_ns} ns")
    return out
